# revision 7
# baseline (speedup 1.0000x reference)
"""Chebyshev graph-conv (gnn_message_passing) Trainium2 kernel.

Reference computation:
    x0 = inputs [1,8,V,8,8,8] -> [V, Fin*B*X*Y*Z]
    Chebyshev recurrence with sparse Laplacian (COO, 8 entries/row), K=5
    out = einsum('kvfbxyz,kfo->bovxyz', cheb, weight) + bias

Sharding: dense dim D = Fin*XYZ split over the XYZ axis across 8 cores
(64 spatial positions per core -> local D = 64*8 = 512, laid out d = s*8+f).

Per-core algorithm:
  - spmv: per v-tile deduped SWDGE dma_gather of x rows in float8_e3m4
    (halves gather HBM bytes vs bf16; E3M4's 4 mantissa bits keep the
    total error ~1e-2 << the 2e-2 budget), folded by PE matmuls with
    bf16 selection matrices into PSUM (edge values + 8-way segment sum).
  - Chebyshev terms 1..3 stay resident in SBUF (bf16); term 0 is
    streamed JIT from DRAM for the k=2 combine (keeps the k=1 window
    free for gathers); term 4 lives in a small per-chunk buffer consumed
    by the immediately-following output stage. The combine
    (x_k = 2*psum - x_{k-2}) runs on DVE; an e3m4 copy of x_1..x_3 goes
    to DRAM (batched per chunk) as the next spmv's gather source.
  - Output einsum accumulates over k in PSUM per (half, ti) [2 bufs for
    drain/matmul overlap]: term 0 arrives pre-transposed via DMA
    transpose-mode gathers from x0b; terms 1..4 are PE-transposed from
    SBUF. Bias added on drain; out stored bf16 (2 planes per DMA) and
    upcast on host.
"""

import sys

for _p in ("/opt/trn_rl_repo", "/root/.axon_site/_ro/trn_rl_repo"):
    if _p not in sys.path:
        sys.path.append(_p)

import numpy as np

V = 2562
DEG = 8
B, FIN, FOUT, K = 1, 8, 16, 5
XYZ = 512
NCORES = 8
SLOC = XYZ // NCORES  # 64 spatial positions per core
D = SLOC * FIN  # 512 local dense dim, d = s_loc*8 + f

VP = 2688  # V padded to 21*128
NT = VP // 128  # 21 v-tiles
EPAD = VP * DEG  # 21504 padded edges
NVCH = 6  # v-chunks of up to 4 v-tiles (5*4 + 1)

_COMPILED = [None]
LAST_RESULT = [None]


def _build(gchs, choffs):
    """gchs[vt] = gathered 128-row groups for v-tile vt; choffs[ch] = group
    offset of chunk ch (choffs[NVCH] = total groups G)."""
    import os as _os
    from contextlib import ExitStack

    import concourse.mybir as mybir
    import concourse.tile as tile
    from concourse import bacc

    _ring = int(_os.environ.get("DMA_RING", "16384"))

    fp32 = mybir.dt.float32
    i16 = mybir.dt.int16
    bf16 = mybir.dt.bfloat16
    f8e3 = mybir.dt.float8e3
    Alu = mybir.AluOpType

    G = choffs[NVCH]
    gch_max = max(gchs)

    nc = bacc.Bacc(
        None, target_bir_lowering=False, dynamic_dma_scratch_size=_ring
    )

    x0b = nc.dram_tensor("x0b", [VP, D], bf16, kind="ExternalInput")
    x0e = nc.dram_tensor("x0e", [VP, D], f8e3, kind="ExternalInput")
    idxd = nc.dram_tensor("idx", [128, G * 8], i16, kind="ExternalInput")
    idxtd = nc.dram_tensor("idxt", [128, 32], i16, kind="ExternalInput")
    seld = nc.dram_tensor("sel", [128, G, 128], bf16, kind="ExternalInput")
    wseld = nc.dram_tensor("wsel", [128, K * 2, 128], bf16, kind="ExternalInput")
    biasd = nc.dram_tensor("biasx", [128, 2], fp32, kind="ExternalInput")
    outd = nc.dram_tensor("outT", [8, 128, VP], bf16, kind="ExternalOutput")

    from concourse.masks import make_identity

    with ExitStack() as ctx:
        tc = ctx.enter_context(tile.TileContext(nc))
        const = ctx.enter_context(tc.tile_pool(name="const", bufs=1))
        chp = ctx.enter_context(tc.tile_pool(name="cheb", bufs=1))
        ch0p = ctx.enter_context(tc.tile_pool(name="ch0", bufs=6))
        ch4p = ctx.enter_context(tc.tile_pool(name="ch4", bufs=2))
        dram = ctx.enter_context(tc.tile_pool(name="dram", bufs=1, space="DRAM"))
        gp = ctx.enter_context(tc.tile_pool(name="g", bufs=6))
        sp = ctx.enter_context(tc.tile_pool(name="stream", bufs=2))
        xtp = ctx.enter_context(tc.tile_pool(name="xt", bufs=3))
        xgp = ctx.enter_context(tc.tile_pool(name="xtg", bufs=2))
        op = ctx.enter_context(tc.tile_pool(name="ob", bufs=4))
        ppv = ctx.enter_context(tc.tile_pool(name="psumv", bufs=2, space="PSUM"))
        ppo = ctx.enter_context(tc.tile_pool(name="psumo", bufs=2, space="PSUM"))
        ppt = ctx.enter_context(tc.tile_pool(name="psumt", bufs=2, space="PSUM"))

        SEL = const.tile([128, G, 128], bf16)
        WSEL = const.tile([128, K * 2, 128], bf16)
        IDX = const.tile([128, G * 8], i16)
        IDXT = const.tile([128, 32], i16)
        BIA = const.tile([128, 2], fp32)
        IDNB = const.tile([128, 128], bf16)
        IDN = const.tile([128, 128], fp32)
        # head-critical consts first: idx, then SEL per-chunk interleaved with
        # the k=1 gathers (issued below inside the k-loop)
        with tc.high_priority(offset=2000):
            nc.sync.dma_start(IDX[:], idxd[:])
        with tc.high_priority(offset=1200):
            nc.sync.dma_start(IDXT[:], idxtd[:])
            nc.sync.dma_start(BIA[:], biasd[:])
            nc.sync.dma_start(WSEL[:], wseld[:])
        make_identity(nc, IDN[:])
        nc.vector.tensor_copy(IDNB[:], IDN[:])

        # resident Chebyshev terms 1..3, bf16 [128, NT, 512]
        CH = [None] + [
            chp.tile([128, NT, D], bf16, tag=f"ch{k}", name=f"ch{k}")
            for k in range(1, K - 1)
        ]
        # e3m4 gather sources for spmv k=2..4 (written on-chip k=1..3)
        xg = [dram.tile([VP, D], f8e3, tag=f"xg{i}", name=f"xg{i}") for i in range(3)]

        def trg(src, ch, n, tag):
            """Transpose-mode gather of chunk ch (n rows) -> [128, 4, n]."""
            t = xgp.tile([128, 4, n], bf16, tag=f"{tag}{n}")
            with tc.high_priority(offset=300):
                nc.gpsimd.dma_gather(
                    t[:, :, :],
                    src[512 * ch : 512 * ch + n, :],
                    IDXT[:, : n // 16],
                    num_idxs=n,
                    num_idxs_reg=n,
                    elem_size=D,
                    transpose=True,
                )
            return t

        def out_stage(ch, nvt, ch4):
            """PSUM-accumulated over k output for chunk ch (nvt v-tiles)."""
            n = nvt * 128
            xT0 = trg(x0b, ch, n, "t0")
            for half in range(2):
                for ti in range(2):
                    t = 2 * half + ti
                    psO = ppo.tile([128, 2, 512], fp32, tag="psO")
                    for h in range(2):
                        nc.tensor.matmul(
                            psO[:, h, :n],
                            WSEL[:, h, :],
                            xT0[:, t, :n],
                            start=True,
                            stop=False,
                        )
                    for k in range(1, K):
                        psT = ppt.tile([128, 512], bf16, tag="psT")
                        for vl in range(nvt):
                            src = (
                                ch4[:, vl, 128 * t : 128 * (t + 1)]
                                if k == K - 1
                                else CH[k][:, 4 * ch + vl, 128 * t : 128 * (t + 1)]
                            )
                            nc.tensor.transpose(
                                psT[:, 128 * vl : 128 * (vl + 1)], src, IDNB[:]
                            )
                        xT = xtp.tile([128, 512], bf16, tag="xT")
                        nc.scalar.copy(xT[:, :n], psT[:, :n])
                        for h in range(2):
                            nc.tensor.matmul(
                                psO[:, h, :n],
                                WSEL[:, k * 2 + h, :],
                                xT[:, :n],
                                start=False,
                                stop=(k == K - 1),
                            )
                    ob = op.tile([128, 2, 512], bf16, tag="ob")
                    nc.vector.tensor_scalar(
                        ob[:, 0, :n],
                        psO[:, 0, :n],
                        BIA[:, 0:1],
                        None,
                        Alu.add,
                    )
                    nc.scalar.activation(
                        ob[:, 1, :n],
                        psO[:, 1, :n],
                        mybir.ActivationFunctionType.Identity,
                        bias=BIA[:, 1:2],
                    )
                    i0 = t * 2
                    nc.sync.dma_start(
                        outd[i0 : i0 + 2, :, 512 * ch : 512 * ch + n].rearrange(
                            "i p v -> p i v"
                        ),
                        ob[:, :, :n],
                    )

        # ---- k = 1..4: x_k = 2 L x_{k-1} - x_{k-2}   (k=1: x_1 = L x_0) ----
        # cheb_0 is never materialized whole on-chip: the k=2 combine streams
        # it JIT from x0b (window of ch0p tiles), the output stage reads it
        # via transpose-gathers from x0b.
        ch0_tiles = {}

        def ch0_load(vt):
            t = ch0p.tile([128, D], bf16, tag="c0")
            nc.sync.dma_start(t[:], x0b[128 * vt : 128 * (vt + 1), :])
            ch0_tiles[vt] = t

        for k in range(1, K):
            src = x0e if k == 1 else xg[k - 2]
            for ch in range(NVCH):
                nvt = 4 if ch < 5 else 1
                if k == 1:
                    # SEL groups for this chunk arrive just ahead of its folds
                    with tc.high_priority(offset=450):
                        nc.sync.dma_start(
                            SEL[:, choffs[ch] : choffs[ch + 1], :],
                            seld[:, choffs[ch] : choffs[ch + 1], :],
                        )
                # gathers issued in reverse tile order: fold(t0) then waits for
                # the whole chunk, so PE runs 4-tile bursts back-to-back (stays
                # out of the low p-states) while the next chunk's gathers land
                gts = [None] * nvt
                goffs = [
                    sum(gchs[4 * ch + j] for j in range(vl)) for vl in range(nvt)
                ]
                for vl in reversed(range(nvt)):
                    vt = 4 * ch + vl
                    nidx = gchs[vt] * 128
                    gt = gp.tile([128, gch_max, D], f8e3, tag="g", name=f"g_{k}_{vt}")
                    with tc.high_priority(offset=400):
                        nc.gpsimd.dma_gather(
                            gt[:, : gchs[vt], :],
                            src[:],
                            IDX[
                                :,
                                8 * (choffs[ch] + goffs[vl]) : 8
                                * (choffs[ch] + goffs[vl] + gchs[vt]),
                            ],
                            num_idxs=nidx,
                            num_idxs_reg=nidx,
                            elem_size=D,
                        )
                    gts[vl] = gt
                if k == 2:
                    # stream cheb_0 tiles for this chunk's combines
                    for vl in range(nvt):
                        ch0_load(4 * ch + vl)
                xe = None
                ch4 = None
                if k < K - 1:
                    xe = sp.tile(
                        [128, 4, D], f8e3, tag="xe", name=f"xe_{k}_{ch}"
                    )
                else:
                    ch4 = ch4p.tile([128, 4, D], bf16, tag="ch4", name=f"ch4_{ch}")
                for vl in range(nvt):
                    vt = 4 * ch + vl
                    goff = sum(gchs[4 * ch + j] for j in range(vl))
                    psV = ppv.tile([128, 512], fp32, tag="psV")
                    for j in range(gchs[vt]):
                        nc.tensor.matmul(
                            psV[:],
                            SEL[:, choffs[ch] + goff + j, :],
                            gts[vl][:, j, :],
                            start=(j == 0),
                            stop=(j == gchs[vt] - 1),
                        )
                    with tc.high_priority(offset=600):
                        if k == K - 1:
                            xv = ch4[:, vl, :]
                        else:
                            xv = CH[k][:, vt, :]
                        if k == 1:
                            nc.vector.tensor_copy(xv, psV[:])
                        else:
                            sub = (
                                ch0_tiles.pop(vt)[:]
                                if k == 2
                                else CH[k - 2][:, vt, :]
                            )
                            nc.vector.scalar_tensor_tensor(
                                xv,
                                psV[:],
                                2.0,
                                sub,
                                Alu.mult,
                                Alu.subtract,
                            )
                        if k < K - 1:
                            nc.vector.tensor_copy(xe[:, vl, :], xv)
                if k < K - 1:
                    with tc.high_priority(offset=500 if ch == NVCH - 1 else 0):
                        nc.sync.dma_start(
                            xg[k - 1][
                                512 * ch : 512 * ch + nvt * 128, :
                            ].rearrange("(vl p) d -> p vl d", p=128),
                            xe[:, :nvt, :],
                        )
                if k == K - 1:
                    out_stage(ch, nvt, ch4)

    nc.compile()
    return nc


def _host_prep(inputs, lap_rows, lap_cols, lap_vals, weight, bias):
    import ml_dtypes

    inputs = np.asarray(inputs, dtype=np.float32)
    lap_rows = np.asarray(lap_rows)
    lap_cols = np.asarray(lap_cols)
    lap_vals = np.asarray(lap_vals, dtype=np.float32)
    weight = np.asarray(weight, dtype=np.float32)
    bias = np.asarray(bias, dtype=np.float32)

    nnz = lap_rows.shape[0]
    order = np.argsort(lap_rows, kind="stable")
    srows = lap_rows[order]
    assert np.array_equal(
        np.repeat(np.arange(V, dtype=srows.dtype), DEG), srows
    ), "expected exactly DEG entries per row"
    e_cols = np.zeros(EPAD, np.int64)
    e_vals = np.zeros(EPAD, np.float32)
    e_cols[:nnz] = lap_cols[order]
    e_vals[:nnz] = lap_vals[order]

    # per-v-tile dedup: gather each unique col once; SEL folds vals and
    # scatters every (unique col -> output v) pair of the tile
    uniq = [np.unique(e_cols[1024 * vt : 1024 * (vt + 1)]) for vt in range(NT)]
    gchs = tuple(max(1, (len(u) + 127) // 128) for u in uniq)
    toffs = np.concatenate([[0], np.cumsum(gchs)]).astype(np.int64)
    G = int(toffs[NT])
    choffs = tuple(int(toffs[min(4 * ch, NT)]) for ch in range(NVCH + 1))

    idx_np = np.zeros((128, G * 8), np.int16)
    sel_np = np.zeros((128, G, 128), np.float32)
    for ch in range(NVCH):
        lo, hi = choffs[ch], choffs[ch + 1]
        gch_ch = hi - lo
        slots = np.zeros(gch_ch * 128, np.int64)
        for vl in range(4 if ch < 5 else 1):
            vt = 4 * ch + vl
            u = uniq[vt]
            base = (toffs[vt] - lo) * 128
            slots[base : base + len(u)] = u
            col2slot = np.zeros(V + 1, np.int64)
            col2slot[u] = np.arange(len(u))
            ecols = e_cols[1024 * vt : 1024 * (vt + 1)]
            evals = e_vals[1024 * vt : 1024 * (vt + 1)]
            sl = col2slot[ecols]  # slot within this tile's groups
            m = np.arange(1024) // DEG  # output row within v-tile
            np.add.at(sel_np, (sl % 128, toffs[vt] + sl // 128, m), evals)
        # wrapped-16 idx layout over the whole chunk's slot list
        w = slots.reshape(gch_ch * 8, 16).T.astype(np.int16)
        idx_np[:, 8 * lo : 8 * hi] = np.tile(w, (8, 1))

    sel_np = sel_np.astype(ml_dtypes.bfloat16)

    # output-stage weight selection: rows p=s_loc*8+f, cols q=s_loc*8+o
    wsel_np = np.zeros((128, K * 2, 128), np.float32)
    sl = np.arange(16)
    for k in range(K):
        for h in range(2):
            for f in range(FIN):
                for o in range(8):
                    wsel_np[sl * 8 + f, k * 2 + h, sl * 8 + o] = weight[k, f, 8 * h + o]
    wsel_np = wsel_np.astype(ml_dtypes.bfloat16)

    bias_np = np.zeros((128, 2), np.float32)
    p = np.arange(128)
    for h in range(2):
        bias_np[p, h] = bias[8 * h + p % 8]

    # transpose-gather chunk-local sequential indices, wrapped-16
    idxt_np = np.tile(
        np.arange(512, dtype=np.int16).reshape(32, 16).T, (8, 1)
    )

    # x0 shards: [V, s, f] per core
    xt = inputs.reshape(FIN, V, XYZ).transpose(1, 2, 0)  # [V, 512, 8]
    x0s = []
    for m in range(NCORES):
        x0m = np.zeros((VP, D), np.float32)
        x0m[:V] = xt[:, SLOC * m : SLOC * (m + 1), :].reshape(V, D)
        x0s.append(x0m)
    return x0s, idx_np, idxt_np, sel_np, wsel_np, bias_np, gchs, choffs


def kernel(inputs, lap_rows, lap_cols, lap_vals, weight, bias):
    import ml_dtypes as _ml

    from concourse.bass_utils import run_bass_kernel_spmd

    x0s, idx_np, idxt_np, sel_np, wsel_np, bias_np, gchs, choffs = _host_prep(
        inputs, lap_rows, lap_cols, lap_vals, weight, bias
    )

    key = (gchs, choffs)
    if _COMPILED[0] is None or _COMPILED[0][0] != key:
        _COMPILED[0] = (key, _build(gchs, choffs))
    nc = _COMPILED[0][1]

    in_maps = [
        {
            "x0b": x0s[m].astype(_ml.bfloat16),
            "x0e": x0s[m].astype(_ml.float8_e3m4),
            "idx": idx_np,
            "idxt": idxt_np,
            "sel": sel_np,
            "wsel": wsel_np,
            "biasx": bias_np,
        }
        for m in range(NCORES)
    ]
    import os

    trace = bool(int(os.environ.get("KERNEL_TRACE", "0")))
    res = run_bass_kernel_spmd(
        nc, in_maps, core_ids=list(range(NCORES)), trace=trace
    )
    LAST_RESULT[0] = res

    # unshard: outT [8=(t,h), 128=(s_loc,o_loc), VP] per core
    parts = []
    for m in range(NCORES):
        r = res.results[m]["outT"].astype(np.float32)  # [8, 128, VP]
        r = r.reshape(4, 2, 16, 8, VP)[:, :, :, :, :V]  # [t, h, sl, ol, v]
        # o = 8h + ol ; s_local_in_core = 16t + sl
        r = r.transpose(1, 3, 4, 0, 2).reshape(FOUT, V, SLOC)  # [o, v, s]
        parts.append(r)
    out = np.concatenate(parts, axis=2)  # [o, v, 512]
    return np.ascontiguousarray(
        out.reshape(1, FOUT, V, 8, 8, 8).astype(np.float32)
    )


# revision 8
# speedup vs baseline: 1.0374x; 1.0374x over previous
"""Chebyshev graph-conv (gnn_message_passing) Trainium2 kernel.

Reference computation:
    x0 = inputs [1,8,V,8,8,8] -> [V, Fin*B*X*Y*Z]
    Chebyshev recurrence with sparse Laplacian (COO, 8 entries/row), K=5
    out = einsum('kvfbxyz,kfo->bovxyz', cheb, weight) + bias

Sharding: dense dim D = Fin*XYZ split over the XYZ axis across 8 cores
(64 spatial positions per core -> local D = 64*8 = 512, laid out d = s*8+f).

Per-core algorithm:
  - spmv: per v-tile deduped SWDGE dma_gather of x rows in float8_e3m4
    (halves gather HBM bytes vs bf16; E3M4's 4 mantissa bits keep the
    total error ~1e-2 << the 2e-2 budget), folded by PE matmuls with
    bf16 selection matrices into PSUM (edge values + 8-way segment sum).
  - Chebyshev terms 1..3 stay resident in SBUF (bf16); term 0 is
    streamed JIT from DRAM for the k=2 combine (keeps the k=1 window
    free for gathers); term 4 lives in a small per-chunk buffer consumed
    by the immediately-following output stage. The combine
    (x_k = 2*psum - x_{k-2}) runs on DVE; an e3m4 copy of x_1..x_3 goes
    to DRAM (batched per chunk) as the next spmv's gather source.
  - Output einsum accumulates over k in PSUM per (half, ti) [2 bufs for
    drain/matmul overlap]: term 0 arrives pre-transposed via DMA
    transpose-mode gathers from x0b; terms 1..4 are PE-transposed from
    SBUF. Bias added on drain; out stored bf16 (2 planes per DMA) and
    upcast on host.
"""

import sys

for _p in ("/opt/trn_rl_repo", "/root/.axon_site/_ro/trn_rl_repo"):
    if _p not in sys.path:
        sys.path.append(_p)

import numpy as np

V = 2562
DEG = 8
B, FIN, FOUT, K = 1, 8, 16, 5
XYZ = 512
NCORES = 8
SLOC = XYZ // NCORES  # 64 spatial positions per core
D = SLOC * FIN  # 512 local dense dim, d = s_loc*8 + f

VP = 2688  # V padded to 21*128
NT = VP // 128  # 21 v-tiles
EPAD = VP * DEG  # 21504 padded edges
NVCH = 6  # v-chunks of up to 4 v-tiles (5*4 + 1)

_COMPILED = [None]
LAST_RESULT = [None]


def _build(gchs, choffs):
    """gchs[vt] = gathered 128-row groups for v-tile vt; choffs[ch] = group
    offset of chunk ch (choffs[NVCH] = total groups G)."""
    import os as _os
    from contextlib import ExitStack

    import concourse.mybir as mybir
    import concourse.tile as tile
    from concourse import bacc

    _ring = int(_os.environ.get("DMA_RING", "16384"))

    fp32 = mybir.dt.float32
    i16 = mybir.dt.int16
    bf16 = mybir.dt.bfloat16
    f8e3 = mybir.dt.float8e3
    Alu = mybir.AluOpType

    G = choffs[NVCH]
    gch_max = max(gchs)

    nc = bacc.Bacc(
        None, target_bir_lowering=False, dynamic_dma_scratch_size=_ring
    )

    x0b = nc.dram_tensor("x0b", [VP, D], bf16, kind="ExternalInput")
    x0e = nc.dram_tensor("x0e", [VP, D], f8e3, kind="ExternalInput")
    idxd = nc.dram_tensor("idx", [128, G * 8], i16, kind="ExternalInput")
    idxtd = nc.dram_tensor("idxt", [128, 32], i16, kind="ExternalInput")
    seld = nc.dram_tensor("sel", [128, G, 128], bf16, kind="ExternalInput")
    wseld = nc.dram_tensor("wsel", [128, K * 2, 128], bf16, kind="ExternalInput")
    biasd = nc.dram_tensor("biasx", [128, 2], fp32, kind="ExternalInput")
    outd = nc.dram_tensor("outT", [8, 128, VP], bf16, kind="ExternalOutput")

    from concourse.masks import make_identity

    with ExitStack() as ctx:
        tc = ctx.enter_context(tile.TileContext(nc))
        const = ctx.enter_context(tc.tile_pool(name="const", bufs=1))
        chp = ctx.enter_context(tc.tile_pool(name="cheb", bufs=1))
        ch0p = ctx.enter_context(tc.tile_pool(name="ch0", bufs=6))
        ch4p = ctx.enter_context(tc.tile_pool(name="ch4", bufs=2))
        dram = ctx.enter_context(tc.tile_pool(name="dram", bufs=1, space="DRAM"))
        gp = ctx.enter_context(tc.tile_pool(name="g", bufs=8))
        sp = ctx.enter_context(tc.tile_pool(name="stream", bufs=2))
        xtp = ctx.enter_context(tc.tile_pool(name="xt", bufs=3))
        xgp = ctx.enter_context(tc.tile_pool(name="xtg", bufs=2))
        op = ctx.enter_context(tc.tile_pool(name="ob", bufs=4))
        ppv = ctx.enter_context(tc.tile_pool(name="psumv", bufs=2, space="PSUM"))
        ppo = ctx.enter_context(tc.tile_pool(name="psumo", bufs=2, space="PSUM"))
        ppt = ctx.enter_context(tc.tile_pool(name="psumt", bufs=2, space="PSUM"))

        SEL = const.tile([128, G, 128], bf16)
        WSEL = const.tile([128, K * 2, 128], bf16)
        IDX = const.tile([128, G * 8], i16)
        IDXT = const.tile([128, 32], i16)
        BIA = const.tile([128, 2], fp32)
        IDNB = const.tile([128, 128], bf16)
        IDN = const.tile([128, 128], fp32)
        # head-critical consts first: idx, then SEL per-chunk interleaved with
        # the k=1 gathers (issued below inside the k-loop)
        with tc.high_priority(offset=2000):
            nc.sync.dma_start(IDX[:], idxd[:])
        with tc.high_priority(offset=1200):
            nc.sync.dma_start(IDXT[:], idxtd[:])
            nc.sync.dma_start(BIA[:], biasd[:])
            nc.sync.dma_start(WSEL[:], wseld[:])
        make_identity(nc, IDN[:])
        nc.vector.tensor_copy(IDNB[:], IDN[:])

        # resident Chebyshev terms 1..3, bf16 [128, NT, 512]
        CH = [None] + [
            chp.tile([128, NT, D], bf16, tag=f"ch{k}", name=f"ch{k}")
            for k in range(1, K - 1)
        ]
        # e3m4 gather sources for spmv k=2..4 (written on-chip k=1..3)
        xg = [dram.tile([VP, D], f8e3, tag=f"xg{i}", name=f"xg{i}") for i in range(3)]

        def trg(src, ch, n, tag):
            """Transpose-mode gather of chunk ch (n rows) -> [128, 4, n]."""
            t = xgp.tile([128, 4, n], bf16, tag=f"{tag}{n}")
            with tc.high_priority(offset=300):
                nc.gpsimd.dma_gather(
                    t[:, :, :],
                    src[512 * ch : 512 * ch + n, :],
                    IDXT[:, : n // 16],
                    num_idxs=n,
                    num_idxs_reg=n,
                    elem_size=D,
                    transpose=True,
                )
            return t

        def out_stage(ch, nvt, ch4):
            """PSUM-accumulated over k output for chunk ch (nvt v-tiles)."""
            n = nvt * 128
            xT0 = trg(x0b, ch, n, "t0")
            for half in range(2):
                for ti in range(2):
                    t = 2 * half + ti
                    psO = ppo.tile([128, 2, 512], fp32, tag="psO")
                    for h in range(2):
                        nc.tensor.matmul(
                            psO[:, h, :n],
                            WSEL[:, h, :],
                            xT0[:, t, :n],
                            start=True,
                            stop=False,
                        )
                    for k in range(1, K):
                        psT = ppt.tile([128, 512], bf16, tag="psT")
                        for vl in range(nvt):
                            src = (
                                ch4[:, vl, 128 * t : 128 * (t + 1)]
                                if k == K - 1
                                else CH[k][:, 4 * ch + vl, 128 * t : 128 * (t + 1)]
                            )
                            nc.tensor.transpose(
                                psT[:, 128 * vl : 128 * (vl + 1)], src, IDNB[:]
                            )
                        xT = xtp.tile([128, 512], bf16, tag="xT")
                        nc.scalar.copy(xT[:, :n], psT[:, :n])
                        for h in range(2):
                            nc.tensor.matmul(
                                psO[:, h, :n],
                                WSEL[:, k * 2 + h, :],
                                xT[:, :n],
                                start=False,
                                stop=(k == K - 1),
                            )
                    ob = op.tile([128, 2, 512], bf16, tag="ob")
                    nc.vector.tensor_scalar(
                        ob[:, 0, :n],
                        psO[:, 0, :n],
                        BIA[:, 0:1],
                        None,
                        Alu.add,
                    )
                    nc.scalar.activation(
                        ob[:, 1, :n],
                        psO[:, 1, :n],
                        mybir.ActivationFunctionType.Identity,
                        bias=BIA[:, 1:2],
                    )
                    i0 = t * 2
                    nc.sync.dma_start(
                        outd[i0 : i0 + 2, :, 512 * ch : 512 * ch + n].rearrange(
                            "i p v -> p i v"
                        ),
                        ob[:, :, :n],
                    )

        # ---- k = 1..4: x_k = 2 L x_{k-1} - x_{k-2}   (k=1: x_1 = L x_0) ----
        # cheb_0 is never materialized whole on-chip: the k=2 combine streams
        # it JIT from x0b (window of ch0p tiles), the output stage reads it
        # via transpose-gathers from x0b.
        ch0_tiles = {}

        def ch0_load(vt):
            t = ch0p.tile([128, D], bf16, tag="c0")
            nc.sync.dma_start(t[:], x0b[128 * vt : 128 * (vt + 1), :])
            ch0_tiles[vt] = t

        for k in range(1, K):
            src = x0e if k == 1 else xg[k - 2]
            for ch in range(NVCH):
                nvt = 4 if ch < 5 else 1
                if k == 1:
                    # SEL groups for this chunk arrive just ahead of its folds
                    with tc.high_priority(offset=450):
                        nc.sync.dma_start(
                            SEL[:, choffs[ch] : choffs[ch + 1], :],
                            seld[:, choffs[ch] : choffs[ch + 1], :],
                        )
                # gathers issued in reverse tile order: fold(t0) then waits for
                # the whole chunk, so PE runs 4-tile bursts back-to-back (stays
                # out of the low p-states) while the next chunk's gathers land
                gts = [None] * nvt
                goffs = [
                    sum(gchs[4 * ch + j] for j in range(vl)) for vl in range(nvt)
                ]
                for vl in reversed(range(nvt)):
                    vt = 4 * ch + vl
                    nidx = gchs[vt] * 128
                    gt = gp.tile([128, gch_max, D], f8e3, tag="g", name=f"g_{k}_{vt}")
                    with tc.high_priority(offset=400):
                        nc.gpsimd.dma_gather(
                            gt[:, : gchs[vt], :],
                            src[:],
                            IDX[
                                :,
                                8 * (choffs[ch] + goffs[vl]) : 8
                                * (choffs[ch] + goffs[vl] + gchs[vt]),
                            ],
                            num_idxs=nidx,
                            num_idxs_reg=nidx,
                            elem_size=D,
                        )
                    gts[vl] = gt
                if k == 2:
                    # stream cheb_0 tiles for this chunk's combines
                    for vl in range(nvt):
                        ch0_load(4 * ch + vl)
                xe = None
                ch4 = None
                if k < K - 1:
                    xe = sp.tile(
                        [128, 4, D], f8e3, tag="xe", name=f"xe_{k}_{ch}"
                    )
                else:
                    ch4 = ch4p.tile([128, 4, D], bf16, tag="ch4", name=f"ch4_{ch}")
                for vl in range(nvt):
                    vt = 4 * ch + vl
                    goff = sum(gchs[4 * ch + j] for j in range(vl))
                    psV = ppv.tile([128, 512], fp32, tag="psV")
                    for j in range(gchs[vt]):
                        nc.tensor.matmul(
                            psV[:],
                            SEL[:, choffs[ch] + goff + j, :],
                            gts[vl][:, j, :],
                            start=(j == 0),
                            stop=(j == gchs[vt] - 1),
                        )
                    with tc.high_priority(offset=600):
                        if k == K - 1:
                            xv = ch4[:, vl, :]
                        else:
                            xv = CH[k][:, vt, :]
                        if k == 1:
                            nc.vector.tensor_copy(xv, psV[:])
                        else:
                            sub = (
                                ch0_tiles.pop(vt)[:]
                                if k == 2
                                else CH[k - 2][:, vt, :]
                            )
                            nc.vector.scalar_tensor_tensor(
                                xv,
                                psV[:],
                                2.0,
                                sub,
                                Alu.mult,
                                Alu.subtract,
                            )
                        if k < K - 1:
                            nc.vector.tensor_copy(xe[:, vl, :], xv)
                if k < K - 1:
                    with tc.high_priority(offset=500 if ch == NVCH - 1 else 0):
                        nc.sync.dma_start(
                            xg[k - 1][
                                512 * ch : 512 * ch + nvt * 128, :
                            ].rearrange("(vl p) d -> p vl d", p=128),
                            xe[:, :nvt, :],
                        )
                if k == K - 1:
                    out_stage(ch, nvt, ch4)

    nc.compile()
    return nc


def _host_prep(inputs, lap_rows, lap_cols, lap_vals, weight, bias):
    import ml_dtypes

    inputs = np.asarray(inputs, dtype=np.float32)
    lap_rows = np.asarray(lap_rows)
    lap_cols = np.asarray(lap_cols)
    lap_vals = np.asarray(lap_vals, dtype=np.float32)
    weight = np.asarray(weight, dtype=np.float32)
    bias = np.asarray(bias, dtype=np.float32)

    nnz = lap_rows.shape[0]
    order = np.argsort(lap_rows, kind="stable")
    srows = lap_rows[order]
    assert np.array_equal(
        np.repeat(np.arange(V, dtype=srows.dtype), DEG), srows
    ), "expected exactly DEG entries per row"
    e_cols = np.zeros(EPAD, np.int64)
    e_vals = np.zeros(EPAD, np.float32)
    e_cols[:nnz] = lap_cols[order]
    e_vals[:nnz] = lap_vals[order]

    # per-v-tile dedup: gather each unique col once; SEL folds vals and
    # scatters every (unique col -> output v) pair of the tile
    uniq = [np.unique(e_cols[1024 * vt : 1024 * (vt + 1)]) for vt in range(NT)]
    gchs = tuple(max(1, (len(u) + 127) // 128) for u in uniq)
    toffs = np.concatenate([[0], np.cumsum(gchs)]).astype(np.int64)
    G = int(toffs[NT])
    choffs = tuple(int(toffs[min(4 * ch, NT)]) for ch in range(NVCH + 1))

    idx_np = np.zeros((128, G * 8), np.int16)
    sel_np = np.zeros((128, G, 128), np.float32)
    for ch in range(NVCH):
        lo, hi = choffs[ch], choffs[ch + 1]
        gch_ch = hi - lo
        slots = np.zeros(gch_ch * 128, np.int64)
        for vl in range(4 if ch < 5 else 1):
            vt = 4 * ch + vl
            u = uniq[vt]
            base = (toffs[vt] - lo) * 128
            slots[base : base + len(u)] = u
            col2slot = np.zeros(V + 1, np.int64)
            col2slot[u] = np.arange(len(u))
            ecols = e_cols[1024 * vt : 1024 * (vt + 1)]
            evals = e_vals[1024 * vt : 1024 * (vt + 1)]
            sl = col2slot[ecols]  # slot within this tile's groups
            m = np.arange(1024) // DEG  # output row within v-tile
            np.add.at(sel_np, (sl % 128, toffs[vt] + sl // 128, m), evals)
        # wrapped-16 idx layout over the whole chunk's slot list
        w = slots.reshape(gch_ch * 8, 16).T.astype(np.int16)
        idx_np[:, 8 * lo : 8 * hi] = np.tile(w, (8, 1))

    sel_np = sel_np.astype(ml_dtypes.bfloat16)

    # output-stage weight selection: rows p=s_loc*8+f, cols q=s_loc*8+o
    wsel_np = np.zeros((128, K * 2, 128), np.float32)
    sl = np.arange(16)
    for k in range(K):
        for h in range(2):
            for f in range(FIN):
                for o in range(8):
                    wsel_np[sl * 8 + f, k * 2 + h, sl * 8 + o] = weight[k, f, 8 * h + o]
    wsel_np = wsel_np.astype(ml_dtypes.bfloat16)

    bias_np = np.zeros((128, 2), np.float32)
    p = np.arange(128)
    for h in range(2):
        bias_np[p, h] = bias[8 * h + p % 8]

    # transpose-gather chunk-local sequential indices, wrapped-16
    idxt_np = np.tile(
        np.arange(512, dtype=np.int16).reshape(32, 16).T, (8, 1)
    )

    # x0 shards: [V, s, f] per core
    xt = inputs.reshape(FIN, V, XYZ).transpose(1, 2, 0)  # [V, 512, 8]
    x0s = []
    for m in range(NCORES):
        x0m = np.zeros((VP, D), np.float32)
        x0m[:V] = xt[:, SLOC * m : SLOC * (m + 1), :].reshape(V, D)
        x0s.append(x0m)
    return x0s, idx_np, idxt_np, sel_np, wsel_np, bias_np, gchs, choffs


def kernel(inputs, lap_rows, lap_cols, lap_vals, weight, bias):
    import ml_dtypes as _ml

    from concourse.bass_utils import run_bass_kernel_spmd

    x0s, idx_np, idxt_np, sel_np, wsel_np, bias_np, gchs, choffs = _host_prep(
        inputs, lap_rows, lap_cols, lap_vals, weight, bias
    )

    key = (gchs, choffs)
    if _COMPILED[0] is None or _COMPILED[0][0] != key:
        _COMPILED[0] = (key, _build(gchs, choffs))
    nc = _COMPILED[0][1]

    in_maps = [
        {
            "x0b": x0s[m].astype(_ml.bfloat16),
            "x0e": x0s[m].astype(_ml.float8_e3m4),
            "idx": idx_np,
            "idxt": idxt_np,
            "sel": sel_np,
            "wsel": wsel_np,
            "biasx": bias_np,
        }
        for m in range(NCORES)
    ]
    import os

    trace = bool(int(os.environ.get("KERNEL_TRACE", "0")))
    res = run_bass_kernel_spmd(
        nc, in_maps, core_ids=list(range(NCORES)), trace=trace
    )
    LAST_RESULT[0] = res

    # unshard: outT [8=(t,h), 128=(s_loc,o_loc), VP] per core
    parts = []
    for m in range(NCORES):
        r = res.results[m]["outT"].astype(np.float32)  # [8, 128, VP]
        r = r.reshape(4, 2, 16, 8, VP)[:, :, :, :, :V]  # [t, h, sl, ol, v]
        # o = 8h + ol ; s_local_in_core = 16t + sl
        r = r.transpose(1, 3, 4, 0, 2).reshape(FOUT, V, SLOC)  # [o, v, s]
        parts.append(r)
    out = np.concatenate(parts, axis=2)  # [o, v, 512]
    return np.ascontiguousarray(
        out.reshape(1, FOUT, V, 8, 8, 8).astype(np.float32)
    )


# revision 9
# speedup vs baseline: 1.1585x; 1.1167x over previous
"""Chebyshev graph-conv (gnn_message_passing) Trainium2 kernel.

Reference computation:
    x0 = inputs [1,8,V,8,8,8] -> [V, Fin*B*X*Y*Z]
    Chebyshev recurrence with sparse Laplacian (COO, 8 entries/row), K=5
    out = einsum('kvfbxyz,kfo->bovxyz', cheb, weight) + bias

Sharding: dense dim D = Fin*XYZ split over the XYZ axis across 8 cores
(64 spatial positions per core -> local D = 64*8 = 512, laid out d = s*8+f).

Per-core algorithm:
  - spmv: per v-tile deduped SWDGE dma_gather of x rows in float8_e3m4
    (halves gather HBM bytes vs bf16; E3M4's 4 mantissa bits keep the
    total error ~1e-2 << the 2e-2 budget), folded by PE matmuls with
    bf16 selection matrices into PSUM (edge values + 8-way segment sum).
  - Chebyshev terms 1..3 stay resident in SBUF (bf16); term 0 is
    streamed JIT from DRAM for the k=2 combine (keeps the k=1 window
    free for gathers); term 4 lives in a small per-chunk buffer consumed
    by the immediately-following output stage. The combine
    (x_k = 2*psum - x_{k-2}) runs on DVE; an e3m4 copy of x_1..x_3 goes
    to DRAM (batched per chunk) as the next spmv's gather source.
  - Output einsum accumulates over k in PSUM per (half, ti) [2 bufs for
    drain/matmul overlap]: term 0 arrives pre-transposed via DMA
    transpose-mode gathers from x0b; terms 1..4 are PE-transposed from
    SBUF. Bias added on drain; out stored bf16 (2 planes per DMA) and
    upcast on host.
"""

import sys

for _p in ("/opt/trn_rl_repo", "/root/.axon_site/_ro/trn_rl_repo"):
    if _p not in sys.path:
        sys.path.append(_p)

import numpy as np

V = 2562
DEG = 8
B, FIN, FOUT, K = 1, 8, 16, 5
XYZ = 512
NCORES = 8
SLOC = XYZ // NCORES  # 64 spatial positions per core
D = SLOC * FIN  # 512 local dense dim, d = s_loc*8 + f

VP = 2688  # V padded to 21*128
NT = VP // 128  # 21 v-tiles
EPAD = VP * DEG  # 21504 padded edges
NVCH = 6  # v-chunks of up to 4 v-tiles (5*4 + 1)

_COMPILED = [None]
LAST_RESULT = [None]


def _build(gchs, choffs):
    """gchs[vt] = gathered 128-row groups for v-tile vt; choffs[ch] = group
    offset of chunk ch (choffs[NVCH] = total groups G)."""
    import os as _os
    from contextlib import ExitStack

    import concourse.mybir as mybir
    import concourse.tile as tile
    from concourse import bacc

    _ring = int(_os.environ.get("DMA_RING", "16384"))

    fp32 = mybir.dt.float32
    i16 = mybir.dt.int16
    bf16 = mybir.dt.bfloat16
    f8e3 = mybir.dt.float8e3
    Alu = mybir.AluOpType

    G = choffs[NVCH]
    gch_max = max(gchs)

    nc = bacc.Bacc(
        None, target_bir_lowering=False, dynamic_dma_scratch_size=_ring
    )

    x0b = nc.dram_tensor("x0b", [VP, D], bf16, kind="ExternalInput")
    x0e = nc.dram_tensor("x0e", [VP, D], f8e3, kind="ExternalInput")
    idxd = nc.dram_tensor("idx", [128, G * 8], i16, kind="ExternalInput")
    idxtd = nc.dram_tensor("idxt", [128, 32], i16, kind="ExternalInput")
    seld = nc.dram_tensor("sel", [128, G, 128], bf16, kind="ExternalInput")
    wseld = nc.dram_tensor("wsel", [128, K * 2, 128], bf16, kind="ExternalInput")
    biasd = nc.dram_tensor("biasx", [128, 2], fp32, kind="ExternalInput")
    outd = nc.dram_tensor("outT", [8, 128, VP], bf16, kind="ExternalOutput")

    from concourse.masks import make_identity

    with ExitStack() as ctx:
        tc = ctx.enter_context(tile.TileContext(nc))
        const = ctx.enter_context(tc.tile_pool(name="const", bufs=1))
        chp = ctx.enter_context(tc.tile_pool(name="cheb", bufs=1))
        ch0p = ctx.enter_context(tc.tile_pool(name="ch0", bufs=6))
        ch4p = ctx.enter_context(tc.tile_pool(name="ch4", bufs=2))
        dram = ctx.enter_context(tc.tile_pool(name="dram", bufs=1, space="DRAM"))
        gp = ctx.enter_context(tc.tile_pool(name="g", bufs=8))
        sp = ctx.enter_context(tc.tile_pool(name="stream", bufs=2))
        xtp = ctx.enter_context(tc.tile_pool(name="xt", bufs=3))
        xgp = ctx.enter_context(tc.tile_pool(name="xtg", bufs=2))
        op = ctx.enter_context(tc.tile_pool(name="ob", bufs=4))
        ppv = ctx.enter_context(tc.tile_pool(name="psumv", bufs=2, space="PSUM"))
        ppo = ctx.enter_context(tc.tile_pool(name="psumo", bufs=2, space="PSUM"))
        ppt = ctx.enter_context(tc.tile_pool(name="psumt", bufs=2, space="PSUM"))

        SEL = const.tile([128, G, 128], bf16)
        WSEL = const.tile([128, K * 2, 128], bf16)
        IDX = const.tile([128, G * 8], i16)
        IDXT = const.tile([128, 32], i16)
        BIA = const.tile([128, 2], fp32)
        IDNB = const.tile([128, 128], bf16)
        IDN = const.tile([128, 128], fp32)
        # head-critical consts first: idx, then SEL per-chunk interleaved with
        # the k=1 gathers (issued below inside the k-loop)
        with tc.high_priority(offset=2000):
            nc.sync.dma_start(IDX[:], idxd[:])
        with tc.high_priority(offset=1200):
            nc.sync.dma_start(IDXT[:], idxtd[:])
            nc.sync.dma_start(BIA[:], biasd[:])
            nc.sync.dma_start(WSEL[:], wseld[:])
        make_identity(nc, IDN[:])
        nc.vector.tensor_copy(IDNB[:], IDN[:])

        # resident Chebyshev terms 1..3, bf16 [128, NT, 512]
        CH = [None] + [
            chp.tile([128, NT, D], bf16, tag=f"ch{k}", name=f"ch{k}")
            for k in range(1, K - 1)
        ]
        # e3m4 gather sources for spmv k=2..4 (written on-chip k=1..3)
        xg = [dram.tile([VP, D], f8e3, tag=f"xg{i}", name=f"xg{i}") for i in range(3)]

        def trg(src, ch, n, tag):
            """Transpose-mode gather of chunk ch (n rows) -> [128, 4, n]."""
            t = xgp.tile([128, 4, n], bf16, tag=f"{tag}{n}")
            with tc.high_priority(offset=300):
                nc.gpsimd.dma_gather(
                    t[:, :, :],
                    src[512 * ch : 512 * ch + n, :],
                    IDXT[:, : n // 16],
                    num_idxs=n,
                    num_idxs_reg=n,
                    elem_size=D,
                    transpose=True,
                )
            return t

        def out_stage(ch, nvt, ch4):
            """PSUM-accumulated over k output for chunk ch (nvt v-tiles)."""
            n = nvt * 128
            xT0 = trg(x0b, ch, n, "t0")
            for half in range(2):
                for ti in range(2):
                    t = 2 * half + ti
                    psO = ppo.tile([128, 2, 512], fp32, tag="psO")
                    for h in range(2):
                        nc.tensor.matmul(
                            psO[:, h, :n],
                            WSEL[:, h, :],
                            xT0[:, t, :n],
                            start=True,
                            stop=False,
                        )
                    for k in range(1, K):
                        psT = ppt.tile([128, 512], bf16, tag="psT")
                        for vl in range(nvt):
                            src = (
                                ch4[:, vl, 128 * t : 128 * (t + 1)]
                                if k == K - 1
                                else CH[k][:, 4 * ch + vl, 128 * t : 128 * (t + 1)]
                            )
                            nc.tensor.transpose(
                                psT[:, 128 * vl : 128 * (vl + 1)], src, IDNB[:]
                            )
                        xT = xtp.tile([128, 512], bf16, tag="xT")
                        nc.scalar.copy(xT[:, :n], psT[:, :n])
                        for h in range(2):
                            nc.tensor.matmul(
                                psO[:, h, :n],
                                WSEL[:, k * 2 + h, :],
                                xT[:, :n],
                                start=False,
                                stop=(k == K - 1),
                            )
                    ob = op.tile([128, 2, 512], bf16, tag="ob")
                    nc.vector.tensor_scalar(
                        ob[:, 0, :n],
                        psO[:, 0, :n],
                        BIA[:, 0:1],
                        None,
                        Alu.add,
                    )
                    nc.scalar.activation(
                        ob[:, 1, :n],
                        psO[:, 1, :n],
                        mybir.ActivationFunctionType.Identity,
                        bias=BIA[:, 1:2],
                    )
                    i0 = t * 2
                    nc.sync.dma_start(
                        outd[i0 : i0 + 2, :, 512 * ch : 512 * ch + n].rearrange(
                            "i p v -> p i v"
                        ),
                        ob[:, :, :n],
                    )

        # ---- k = 1..4: x_k = 2 L x_{k-1} - x_{k-2}   (k=1: x_1 = L x_0) ----
        # cheb_0 is never materialized whole on-chip: the k=2 combine streams
        # it JIT from x0b (window of ch0p tiles), the output stage reads it
        # via transpose-gathers from x0b.
        ch0_tiles = {}

        def ch0_load(vt):
            t = ch0p.tile([128, D], bf16, tag="c0")
            nc.sync.dma_start(t[:], x0b[128 * vt : 128 * (vt + 1), :])
            ch0_tiles[vt] = t

        for k in range(1, K):
            src = x0e if k == 1 else xg[k - 2]
            for ch in range(NVCH):
                nvt = 4 if ch < 5 else 1
                if k == 1:
                    # SEL groups for this chunk arrive just ahead of its folds
                    with tc.high_priority(offset=450):
                        nc.sync.dma_start(
                            SEL[:, choffs[ch] : choffs[ch + 1], :],
                            seld[:, choffs[ch] : choffs[ch + 1], :],
                        )
                gts = [None] * nvt
                goffs = [
                    sum(gchs[4 * ch + j] for j in range(vl)) for vl in range(nvt)
                ]
                for vl in range(nvt):
                    vt = 4 * ch + vl
                    nidx = gchs[vt] * 128
                    gt = gp.tile([128, gch_max, D], f8e3, tag="g", name=f"g_{k}_{vt}")
                    with tc.high_priority(offset=400):
                        nc.gpsimd.dma_gather(
                            gt[:, : gchs[vt], :],
                            src[:],
                            IDX[
                                :,
                                8 * (choffs[ch] + goffs[vl]) : 8
                                * (choffs[ch] + goffs[vl] + gchs[vt]),
                            ],
                            num_idxs=nidx,
                            num_idxs_reg=nidx,
                            elem_size=D,
                        )
                    gts[vl] = gt
                if k == 2:
                    # stream cheb_0 tiles for this chunk's combines
                    for vl in range(nvt):
                        ch0_load(4 * ch + vl)
                xe = None
                ch4 = None
                if k < K - 1:
                    xe = sp.tile(
                        [128, 4, D], f8e3, tag="xe", name=f"xe_{k}_{ch}"
                    )
                else:
                    ch4 = ch4p.tile([128, 4, D], bf16, tag="ch4", name=f"ch4_{ch}")
                for vl in range(nvt):
                    vt = 4 * ch + vl
                    goff = sum(gchs[4 * ch + j] for j in range(vl))
                    psV = ppv.tile([128, 512], fp32, tag="psV")
                    for j in range(gchs[vt]):
                        nc.tensor.matmul(
                            psV[:],
                            SEL[:, choffs[ch] + goff + j, :],
                            gts[vl][:, j, :],
                            start=(j == 0),
                            stop=(j == gchs[vt] - 1),
                        )
                    with tc.high_priority(offset=600):
                        if k == K - 1:
                            xv = ch4[:, vl, :]
                        else:
                            xv = CH[k][:, vt, :]
                        if k == 1:
                            nc.vector.tensor_copy(xv, psV[:])
                        else:
                            sub = (
                                ch0_tiles.pop(vt)[:]
                                if k == 2
                                else CH[k - 2][:, vt, :]
                            )
                            nc.vector.scalar_tensor_tensor(
                                xv,
                                psV[:],
                                2.0,
                                sub,
                                Alu.mult,
                                Alu.subtract,
                            )
                        if k < K - 1:
                            nc.vector.tensor_copy(xe[:, vl, :], xv)
                if k < K - 1:
                    with tc.high_priority(offset=500 if ch == NVCH - 1 else 0):
                        nc.sync.dma_start(
                            xg[k - 1][
                                512 * ch : 512 * ch + nvt * 128, :
                            ].rearrange("(vl p) d -> p vl d", p=128),
                            xe[:, :nvt, :],
                        )
                if k == K - 1:
                    out_stage(ch, nvt, ch4)

    nc.compile()
    return nc


def _host_prep(inputs, lap_rows, lap_cols, lap_vals, weight, bias):
    import ml_dtypes

    inputs = np.asarray(inputs, dtype=np.float32)
    lap_rows = np.asarray(lap_rows)
    lap_cols = np.asarray(lap_cols)
    lap_vals = np.asarray(lap_vals, dtype=np.float32)
    weight = np.asarray(weight, dtype=np.float32)
    bias = np.asarray(bias, dtype=np.float32)

    nnz = lap_rows.shape[0]
    order = np.argsort(lap_rows, kind="stable")
    srows = lap_rows[order]
    assert np.array_equal(
        np.repeat(np.arange(V, dtype=srows.dtype), DEG), srows
    ), "expected exactly DEG entries per row"
    e_cols = np.zeros(EPAD, np.int64)
    e_vals = np.zeros(EPAD, np.float32)
    e_cols[:nnz] = lap_cols[order]
    e_vals[:nnz] = lap_vals[order]

    # per-v-tile dedup: gather each unique col once; SEL folds vals and
    # scatters every (unique col -> output v) pair of the tile
    uniq = [np.unique(e_cols[1024 * vt : 1024 * (vt + 1)]) for vt in range(NT)]
    gchs = tuple(max(1, (len(u) + 127) // 128) for u in uniq)
    toffs = np.concatenate([[0], np.cumsum(gchs)]).astype(np.int64)
    G = int(toffs[NT])
    choffs = tuple(int(toffs[min(4 * ch, NT)]) for ch in range(NVCH + 1))

    idx_np = np.zeros((128, G * 8), np.int16)
    sel_np = np.zeros((128, G, 128), np.float32)
    for ch in range(NVCH):
        lo, hi = choffs[ch], choffs[ch + 1]
        gch_ch = hi - lo
        slots = np.zeros(gch_ch * 128, np.int64)
        for vl in range(4 if ch < 5 else 1):
            vt = 4 * ch + vl
            u = uniq[vt]
            base = (toffs[vt] - lo) * 128
            slots[base : base + len(u)] = u
            col2slot = np.zeros(V + 1, np.int64)
            col2slot[u] = np.arange(len(u))
            ecols = e_cols[1024 * vt : 1024 * (vt + 1)]
            evals = e_vals[1024 * vt : 1024 * (vt + 1)]
            sl = col2slot[ecols]  # slot within this tile's groups
            m = np.arange(1024) // DEG  # output row within v-tile
            np.add.at(sel_np, (sl % 128, toffs[vt] + sl // 128, m), evals)
        # wrapped-16 idx layout over the whole chunk's slot list
        w = slots.reshape(gch_ch * 8, 16).T.astype(np.int16)
        idx_np[:, 8 * lo : 8 * hi] = np.tile(w, (8, 1))

    sel_np = sel_np.astype(ml_dtypes.bfloat16)

    # output-stage weight selection: rows p=s_loc*8+f, cols q=s_loc*8+o
    wsel_np = np.zeros((128, K * 2, 128), np.float32)
    sl = np.arange(16)
    for k in range(K):
        for h in range(2):
            for f in range(FIN):
                for o in range(8):
                    wsel_np[sl * 8 + f, k * 2 + h, sl * 8 + o] = weight[k, f, 8 * h + o]
    wsel_np = wsel_np.astype(ml_dtypes.bfloat16)

    bias_np = np.zeros((128, 2), np.float32)
    p = np.arange(128)
    for h in range(2):
        bias_np[p, h] = bias[8 * h + p % 8]

    # transpose-gather chunk-local sequential indices, wrapped-16
    idxt_np = np.tile(
        np.arange(512, dtype=np.int16).reshape(32, 16).T, (8, 1)
    )

    # x0 shards: [V, s, f] per core
    xt = inputs.reshape(FIN, V, XYZ).transpose(1, 2, 0)  # [V, 512, 8]
    x0s = []
    for m in range(NCORES):
        x0m = np.zeros((VP, D), np.float32)
        x0m[:V] = xt[:, SLOC * m : SLOC * (m + 1), :].reshape(V, D)
        x0s.append(x0m)
    return x0s, idx_np, idxt_np, sel_np, wsel_np, bias_np, gchs, choffs


def kernel(inputs, lap_rows, lap_cols, lap_vals, weight, bias):
    import ml_dtypes as _ml

    from concourse.bass_utils import run_bass_kernel_spmd

    x0s, idx_np, idxt_np, sel_np, wsel_np, bias_np, gchs, choffs = _host_prep(
        inputs, lap_rows, lap_cols, lap_vals, weight, bias
    )

    key = (gchs, choffs)
    if _COMPILED[0] is None or _COMPILED[0][0] != key:
        _COMPILED[0] = (key, _build(gchs, choffs))
    nc = _COMPILED[0][1]

    in_maps = [
        {
            "x0b": x0s[m].astype(_ml.bfloat16),
            "x0e": x0s[m].astype(_ml.float8_e3m4),
            "idx": idx_np,
            "idxt": idxt_np,
            "sel": sel_np,
            "wsel": wsel_np,
            "biasx": bias_np,
        }
        for m in range(NCORES)
    ]
    import os

    trace = bool(int(os.environ.get("KERNEL_TRACE", "0")))
    res = run_bass_kernel_spmd(
        nc, in_maps, core_ids=list(range(NCORES)), trace=trace
    )
    LAST_RESULT[0] = res

    # unshard: outT [8=(t,h), 128=(s_loc,o_loc), VP] per core
    parts = []
    for m in range(NCORES):
        r = res.results[m]["outT"].astype(np.float32)  # [8, 128, VP]
        r = r.reshape(4, 2, 16, 8, VP)[:, :, :, :, :V]  # [t, h, sl, ol, v]
        # o = 8h + ol ; s_local_in_core = 16t + sl
        r = r.transpose(1, 3, 4, 0, 2).reshape(FOUT, V, SLOC)  # [o, v, s]
        parts.append(r)
    out = np.concatenate(parts, axis=2)  # [o, v, 512]
    return np.ascontiguousarray(
        out.reshape(1, FOUT, V, 8, 8, 8).astype(np.float32)
    )
